# revision 2
# baseline (speedup 1.0000x reference)
"""Trainium2 Bass kernel for nn_CosSim_Loss.

Computes mean of per-batch cosine-similarity Gram matrices of
pred [32, 8, 512, 512] -> scalar.

Strategy: shard the contraction dim L = 512*512 = 262144 across the 8
cores (each core gets L/8 = 32768 contiguous elements of every row).
Each core computes the partial Gram sums D[m, n] = sum_l x[m, l] x[n, l]
for the two 128-row groups (rows = 32 batches x 8 maps = 256) with
TensorE matmuls (contraction on partitions, fp32->bf16 cast during the
DMA load), accumulating in PSUM over 256 k-chunks. The host sums the
8 per-core partial Grams, extracts the per-batch 8x8 diagonal blocks,
normalizes by the row norms (taken from the Gram diagonal) and takes
the mean, with the diagonal forced to exactly 1.0 like the reference.

The data is fed to each core pre-transposed ([p, t, m] with l-chunk on
partitions) so the device DMAs are dense 16 KiB/partition descriptors
and no on-chip transpose is needed; the hardware still reads the full
256 MiB of fp32 input.
"""

import os
import sys

import numpy as np

for _p in ("/opt/trn_rl_repo", "/root/.axon_site/_ro/trn_rl_repo"):
    if os.path.isdir(_p) and _p not in sys.path:
        sys.path.append(_p)

import concourse.bass as bass  # noqa: E402
import concourse.mybir as mybir  # noqa: E402
from concourse import bacc  # noqa: E402
from concourse.bass_utils import run_bass_kernel_spmd  # noqa: E402
from concourse.tile import TileContext  # noqa: E402

N_CORES = 8
B, NMAP, H, W = 32, 8, 512, 512
L = H * W  # 262144
ROWS = B * NMAP  # 256
L_SHARD = L // N_CORES  # 32768
T_PER_CORE = L_SHARD // 128  # 256
EPS = 1e-8
NBLK = 16  # t-chunks per DMA (2 MiB fp32 read -> 1 MiB bf16 in SBUF)

_nc_cache = {}


def build_nc(t_per_core=T_PER_CORE, nblk=NBLK):
    """Build + compile the per-core Bass program (same program on all cores)."""
    key = (t_per_core, nblk)
    if key in _nc_cache:
        return _nc_cache[key]

    nc = bacc.Bacc(None, target_bir_lowering=False, debug=False)
    xt = nc.dram_tensor(
        "xt", [128, t_per_core, ROWS], mybir.dt.float32, kind="ExternalInput"
    )
    gram = nc.dram_tensor("gram", [128, 256], mybir.dt.float32, kind="ExternalOutput")

    with TileContext(nc) as tc:
        with (
            tc.tile_pool(name="load", bufs=4) as lp,
            tc.tile_pool(name="psum", bufs=1, space=bass.MemorySpace.PSUM) as pp,
            tc.tile_pool(name="outp", bufs=1) as op,
        ):
            ps = [
                pp.tile([128, 128], mybir.dt.float32, name=f"ps{g}", tag=f"ps{g}")
                for g in range(2)
            ]
            n_blocks = t_per_core // nblk
            for blk in range(n_blocks):
                bt = lp.tile([128, nblk, ROWS], mybir.dt.bfloat16, tag="bt")
                # gpsimd (SWDGE) DMA casts fp32 -> bf16 inline
                nc.gpsimd.dma_start(
                    out=bt[:], in_=xt[:, blk * nblk : (blk + 1) * nblk, :]
                )
                for tl in range(nblk):
                    t = blk * nblk + tl
                    for g in range(2):
                        sl = bt[:, tl, g * 128 : (g + 1) * 128]
                        nc.tensor.matmul(
                            ps[g],
                            sl,
                            sl,
                            start=(t == 0),
                            stop=(t == t_per_core - 1),
                        )
            outt = op.tile([128, 256], mybir.dt.float32, tag="outt")
            for g in range(2):
                nc.vector.tensor_copy(
                    out=outt[:, g * 128 : (g + 1) * 128], in_=ps[g]
                )
            nc.sync.dma_start(out=gram[:], in_=outt[:])

    nc.compile()
    _nc_cache[key] = nc
    return nc


def shard_inputs(pred):
    """[32, 8, 512, 512] fp32 -> per-core [128, T_PER_CORE, 256] arrays.

    Per-core layout: xt[p, t, m] = x[m, c*32768 + t*128 + p] where
    x = pred.reshape(256, 262144). Done in cache-friendly stages.
    """
    x = np.ascontiguousarray(pred, dtype=np.float32).reshape(ROWS, L // 128, 128)
    # stage 1: [m, T, p] -> [T, m, p]   (inner 512B runs are contiguous)
    g = np.ascontiguousarray(x.transpose(1, 0, 2))
    # stage 2: [T, m, p] -> [T, p, m]   (per-T 128 KiB slice, cache resident)
    h = np.ascontiguousarray(g.transpose(0, 2, 1))
    # stage 3: [c*t, p, m] -> [c, p, t, m]  (inner 1 KiB contiguous runs)
    xt = np.ascontiguousarray(
        h.reshape(N_CORES, T_PER_CORE, 128, ROWS).transpose(0, 2, 1, 3)
    )
    return xt


def postprocess(gram_list):
    """Sum per-core partial Grams and reduce to the scalar loss."""
    d = np.zeros((128, 256), dtype=np.float64)
    for garr in gram_list:
        d += np.asarray(garr, dtype=np.float64)
    total = 0.0
    for b in range(B):
        g, j = divmod(b, 16)
        blk = d[8 * j : 8 * j + 8, g * 128 + 8 * j : g * 128 + 8 * j + 8]
        norms = np.sqrt(np.maximum(np.diag(blk), 0.0))
        denom = np.maximum(norms, EPS)
        gn = blk / np.outer(denom, denom)
        np.fill_diagonal(gn, 1.0)
        total += gn.sum()
    return np.asarray(total / (B * NMAP * NMAP), dtype=np.float32)


def run(pred, trace=False, **spmd_kwargs):
    pred = np.asarray(pred, dtype=np.float32)
    assert pred.shape == (B, NMAP, H, W), pred.shape
    nc = build_nc()
    xt = shard_inputs(pred)
    in_maps = [{"xt": xt[c]} for c in range(N_CORES)]
    res = run_bass_kernel_spmd(
        nc, in_maps, core_ids=list(range(N_CORES)), trace=trace, **spmd_kwargs
    )
    value = postprocess([r["gram"] for r in res.results])
    return value, res


def kernel(pred):
    value, _ = run(pred, trace=False)
    return value
